# revision 25
# baseline (speedup 1.0000x reference)
"""Chamfer-distance (nn_CD_loss) Trainium2 kernel — single-pass D design.

Computes reference:
    p1 = pixel2xyz(target), p2 = pixel2xyz(pred)   (N=16384 points each)
    D[i,j] = |p1_i|^2 + |p2_j|^2 - 2 p1_i.p2_j
    m12 = mean over valid i of min over valid j of D[i,j]
    m21 = mean over valid j of min over valid i of D[i,j]
    return m12 + m21

Strategy (8 NeuronCores, SPMD), v2 — one D matrix, both reductions:
  Each core owns a 2048-row stripe of the SINGLE distance matrix
  (rows = its p1 slice, cols = all 16384 p2 points).  The GEMM carries
  BOTH squared-norm terms (K=30: 24 bf16-split product rows + 3 rows of
  -sq2m[j] against ones + 3 rows of -sq1m[i] against ones), so PSUM
  holds -D[i,j] <= 0 directly; near the row/col maxima the values are
  ~-dist (small), which makes bf16 staging precision-safe: only
  near-ties (within ~0.4%) can flip the argmax, changing the result by
  <0.4% against a 2e-2 gate.
  Per [128,2048] PSUM chunk:
    ACT stages fp32 -> SBUF bf16 (~0.92 ns/elem incl overhead).
    DVE (bf16 tensor ops run in 2x_1p mode, 0.52 ns/elem):
      rowacc[128,2048]  = max(rowacc, chunk)   (fold over the 8 chunks)
      colacc[:, chunk]  = max(colacc, chunk)   (fold over the 16 blocks)
    (chunk 0 of a block / block 0 of a column use tensor_copy, 4x mode,
     which also re-initializes the accumulators each repeat iteration.)
  Per block: one tensor_reduce(max) of rowacc -> -dist12 for 128 rows.
  End: colacc [128,16384] is transposed 128x128 at a time on the PE
  (bf16 transpose -> PSUM bf16), and DVE tensor_reduce(max) over the
  transposed free axis folds the partition direction -> per-column
  -min over this core's 2048 rows; host takes the max across cores.
  This does the whole job with ONE pass over D (the old kernel built
  D and D^T separately): PE work halves, and the reduce path drops from
  (1 ScalarE copy + 1 DVE fp32 max)/elem x 2 directions to
  (1 ACT copy + 2 bf16-2x DVE ops)/elem x 1 direction.
"""

import numpy as np
import ml_dtypes

import concourse.bacc as bacc
import concourse.bass_isa as bass_isa
import concourse.mybir as mybir
import concourse.tile as tile
from concourse.bass_utils import run_bass_kernel_spmd

H = W = 128
N = H * W                  # 16384 points per cloud
NCORES = 8
SHARE = N // NCORES        # 2048 rows per core
BLOCKS = SHARE // 128      # 16 row-blocks of 128
K = 30                     # 24 product rows + 3 (-sq2m) rows + 3 (-sq1m) rows
CHUNK = 2048               # psum tile free size (4 banks)
NCHUNK = N // CHUNK        # 8 chunks per block row
INF = np.float32(1.0e30)

_BF16 = ml_dtypes.bfloat16
# (lhs split level, rhs split level); 0=hi 1=mid 2=lo.  All 9 except (2,2).
_GROUPS = [(0, 0), (0, 1), (1, 0), (0, 2), (2, 0), (1, 1), (1, 2), (2, 1)]


def _pixel2xyz(depth, P):
    """depth [1,1,H,W] fp32 -> [N,3] fp32 (mirrors reference._pixel2xyz)."""
    d = depth[0, 0]
    px = np.broadcast_to(np.arange(W, dtype=np.float32)[None, :], (H, W))
    py = np.broadcast_to(np.arange(H, dtype=np.float32)[:, None], (H, W))
    c_u, c_v, f_u, f_v = P[0, 2], P[1, 2], P[0, 0], P[1, 1]
    x = (px * (d + P[2, 3]) - (c_u * d + P[0, 3])) / f_u
    y = (py * (d + P[2, 3]) - (c_v * d + P[1, 3])) / f_v
    return np.stack((x, y, d), axis=-1).reshape(-1, 3).astype(np.float32)


def _split3(v):
    """Exact-ish 3-way bf16 split of fp32 array: v ~= h + m + l."""
    h = v.astype(_BF16)
    r = v - h.astype(np.float32)
    m = r.astype(_BF16)
    r2 = r - m.astype(np.float32)
    l = r2.astype(_BF16)
    return h, m, l


def _lhs_emb(Q, sq_masked):
    """Stationary-side embedding [K, n]: split3(2Q) products + ones + -sq1m."""
    s = _split3(2.0 * Q)                       # each [n,3]
    rows = [s[a][:, c] for (a, _) in _GROUPS for c in range(3)]
    rows += [np.full(Q.shape[0], -1.0, dtype=_BF16)] * 3   # pair with sq2m rows
    rows += list(_split3(sq_masked))                        # pair with ones rows
    return np.stack(rows, axis=0)              # [30, n]


def _rhs_emb(R, sq_masked):
    """Moving-side embedding [K, n]: split3(R) products + sq2m + ones."""
    t = _split3(R)
    rows = [t[b][:, c] for (_, b) in _GROUPS for c in range(3)]
    rows += list(_split3(sq_masked))
    rows += [np.full(R.shape[0], -1.0, dtype=_BF16)] * 3
    return np.stack(rows, axis=0)              # [30, n]


def build_program(reps=1, staged_bufs=4):
    """Build + compile the SPMD single-core program (same NEFF on all 8 cores).

    For reps>1 (timing builds) the body is unrolled 2x inside the hardware
    loop with ping-ponged column accumulators, so the Pool-engine
    partition_all_reduce over colacc of one phase overlaps the next phase's
    GEMM/fold instead of blocking its colacc writes.
    """
    nc = bacc.Bacc("TRN2", target_bir_lowering=False, debug=False,
                   num_devices=NCORES)
    f32 = mybir.dt.float32
    bf16 = mybir.dt.bfloat16
    A = mybir.AluOpType

    lhs = nc.dram_tensor("lhs", [K, SHARE], bf16, kind="ExternalInput")
    rhs = nc.dram_tensor("rhs", [K, N], bf16, kind="ExternalInput")
    minrow = nc.dram_tensor("minrow", [128, BLOCKS], f32, kind="ExternalOutput")
    colout = nc.dram_tensor("colout", [1, N], bf16, kind="ExternalOutput")

    with tile.TileContext(nc) as tc:
        with (
            tc.tile_pool(name="const", bufs=1) as cpool,
            tc.tile_pool(name="psum", bufs=2, space="PSUM") as ppool,
            tc.tile_pool(name="staged", bufs=staged_bufs) as spool,
            tc.tile_pool(name="rowacc", bufs=2) as rpool,
        ):
            lhs_sb = cpool.tile([K, SHARE], bf16, tag="lhs")
            rhs_sb = cpool.tile([K, N], bf16, tag="rhs")
            colaccs = [cpool.tile([128, N], bf16, tag="colaccA", name="colaccA")]
            if reps > 1:
                colaccs.append(cpool.tile([128, N], bf16, tag="colaccB", name="colaccB"))
            colmin_sb = cpool.tile([128, N], bf16, tag="colmin")
            minrow_sb = cpool.tile([128, BLOCKS], f32, tag="minrow")
            nc.sync.dma_start(lhs_sb[:], lhs[:])
            for d0 in range(0, N, 4096):
                nc.sync.dma_start(rhs_sb[:, d0:d0 + 4096],
                                  rhs[:, d0:d0 + 4096])

            def phase(colacc):
                X = mybir.AxisListType.X
                for b in range(BLOCKS):
                    lhs_blk = lhs_sb[:, b * 128:(b + 1) * 128]
                    rowacc = rpool.tile([128, CHUNK], bf16, tag="ra",
                                        name="rowacc")
                    for q in range(NCHUNK):
                        base = q * CHUNK
                        pe_t = ppool.tile([128, CHUNK], f32, tag="ps")
                        for g in range(CHUNK // 512):
                            c0 = base + g * 512
                            nc.tensor.matmul(
                                pe_t[:, g * 512:(g + 1) * 512], lhs_blk,
                                rhs_sb[:, c0:c0 + 512], start=True, stop=True)
                        sb_t = spool.tile([128, CHUNK], bf16, tag="st")
                        nc.scalar.copy(sb_t[:], pe_t[:])
                        # row fold over chunks (TT runs 2x in bf16)
                        if q == 0:
                            nc.vector.tensor_copy(rowacc[:], sb_t[:])
                        else:
                            nc.vector.tensor_tensor(
                                out=rowacc[:], in0=sb_t[:], in1=rowacc[:],
                                op=A.max)
                        # column fold over blocks (TT runs 2x in bf16)
                        cslice = colacc[:, base:base + CHUNK]
                        if b == 0:
                            nc.vector.tensor_copy(cslice, sb_t[:])
                        else:
                            nc.vector.tensor_tensor(
                                out=cslice, in0=sb_t[:], in1=cslice, op=A.max)
                    nc.vector.tensor_reduce(
                        minrow_sb[:, b:b + 1], rowacc[:], axis=X, op=A.max)
                # fold colacc's partition axis on the (otherwise idle) Pool
                # engine; the all-reduce broadcasts the result to every
                # partition, so DMA out row 0 only.
                nc.gpsimd.partition_all_reduce(
                    colmin_sb[:], colacc[:], channels=128,
                    reduce_op=bass_isa.ReduceOp.max)
                nc.sync.dma_start(minrow[:], minrow_sb[:])
                nc.sync.dma_start(colout[:], colmin_sb[0:1, :])

            if reps > 1:
                with tc.For_i(0, reps // 2, 1,
                              hint_engines=(mybir.EngineType.PE,)):
                    phase(colaccs[0])
                    phase(colaccs[1])
            else:
                phase(colaccs[0])
    nc.compile()
    return nc


def host_prep(pred, target, P_rect):
    pred = np.asarray(pred, dtype=np.float32)
    target = np.asarray(target, dtype=np.float32)
    P_rect = np.asarray(P_rect, dtype=np.float32)
    p1 = _pixel2xyz(target, P_rect)
    p2 = _pixel2xyz(pred, P_rect)
    valid = (target[0] > 0).reshape(-1)
    sq1 = np.sum(p1 * p1, axis=1).astype(np.float32)
    sq2 = np.sum(p2 * p2, axis=1).astype(np.float32)
    sq1m = np.where(valid, sq1, INF).astype(np.float32)
    sq2m = np.where(valid, sq2, INF).astype(np.float32)
    lhs = np.ascontiguousarray(_lhs_emb(p1, sq1m))   # stationary: p1 rows
    rhs = np.ascontiguousarray(_rhs_emb(p2, sq2m))   # moving: all p2
    return valid, lhs, rhs


def make_in_maps(lhs, rhs):
    in_maps = []
    for c in range(NCORES):
        sl = slice(c * SHARE, (c + 1) * SHARE)
        in_maps.append({
            "lhs": np.ascontiguousarray(lhs[:, sl]),
            "rhs": rhs,
        })
    return in_maps


def finalize(results, valid):
    # minrow[c][p, b] = max_j -D[i,j] = -dist12[i],  i = c*2048 + b*128 + p
    dist12 = -np.concatenate(
        [np.asarray(results[c]["minrow"]).T.reshape(-1) for c in range(NCORES)]
    ).astype(np.float64)
    # colout[c][0, j] = max over core c's rows of -D[., j]
    percore = np.stack(
        [np.asarray(results[c]["colout"]).reshape(-1).astype(np.float32)
         for c in range(NCORES)])
    dist21 = -percore.max(axis=0).astype(np.float64)
    n = float(valid.sum())
    m12 = dist12[valid].sum() / n
    m21 = dist21[valid].sum() / n
    return np.asarray(np.float32(m12 + m21))


def kernel(pred, target, P_rect):
    valid, lhs, rhs = host_prep(pred, target, P_rect)
    nc = build_program()
    in_maps = make_in_maps(lhs, rhs)
    try:
        res = run_bass_kernel_spmd(nc, in_maps, core_ids=list(range(NCORES)))
    except ModuleNotFoundError:
        # BASS_TRACE set but the axon NTFF hook is unavailable in this
        # environment; retry with tracing hard-disabled.
        import os
        os.environ["BASS_NEVER_TRACE"] = "1"
        res = run_bass_kernel_spmd(nc, in_maps, core_ids=list(range(NCORES)))
    return finalize(res.results, valid)
